# revision 3
# baseline (speedup 1.0000x reference)
"""Trainium2 Bass kernel for nn_CombineRadialSpeciesWithAngular.

Per-angular-order GEMM out_l = v_l @ W[l], flattened+concatenated over l.
Full shapes: v_l [20000, 2l+1, 128] f32 (l=0..5), W [6, 128, 256] f32,
out [720000, 256] f32.

Strategy (8 NeuronCores, data-parallel over samples):
  - Each core gets 2500 samples of every block -> 90000 output rows.
  - Host pre-transposes each core's rows into vt [128, 90000] (contraction
    dim p on partitions) so the device does zero transposes; W is
    rearranged to [128, 6, 256] and replicated.
  - Device: 720 matmuls per core (125-row chunks; 2500 = 20*125 so chunks
    never straddle an angular-block boundary), stationary = vt chunk
    [128,125], moving = W[l] [128,256], float32r PE path (1 cyc/row,
    ~1.3e-4 rel err), PSUM -> SBUF via DVE, 2.56 MB output DMAs.
  - The kernel is DMA-bound: ~138 MB/core at ~360 GB/s.

Uses bacc.Bacc (not bass.Bass): its compile pipeline legalizes semaphore
waits to this target's 1-wait-per-instruction limit; plain Bass output
fails walrus codegen ("Too many sync wait commands").
"""

import math
import sys

import numpy as np

for _p in ("/opt/trn_rl_repo", "/root/.axon_site/_ro/trn_rl_repo"):
    if _p not in sys.path:
        sys.path.append(_p)

import concourse.bacc as bacc
import concourse.mybir as mybir
import concourse.tile as tile
from concourse.bass_utils import run_bass_kernel_spmd

N_CORES = 8
N_SAMPLES = 20000
N_PROPS = 128
N_COMB = 256
N_ANG = 6
S_CORE = N_SAMPLES // N_CORES          # 2500 samples per core
M_TOTAL = sum(2 * l + 1 for l in range(N_ANG))  # 36
ROWS = S_CORE * M_TOTAL                # 90000 rows per core
CHUNK = 125                            # rows per matmul (2500 = 20*125)
G = 20                                 # chunks per supertile (= 2500 rows)
NSUP = ROWS // (G * CHUNK)             # 36 supertiles
GROUP = 4                              # chunks per PSUM tile (2 banks)
NGROUP = G // GROUP                    # 5

F32 = mybir.dt.float32
F32R = mybir.dt.float32r

_nc_cache = {}


def build_nc():
    if "nc" in _nc_cache:
        return _nc_cache["nc"]

    nc = bacc.Bacc()
    vt = nc.dram_tensor("vt", [128, ROWS], F32R, kind="ExternalInput")
    w = nc.dram_tensor("w", [128, N_ANG, N_COMB], F32R, kind="ExternalInput")
    out = nc.dram_tensor("out", [ROWS, N_COMB], F32, kind="ExternalOutput")
    out_v = out.rearrange("(s j r) c -> s r j c", s=NSUP, j=G, r=CHUNK)

    with tile.TileContext(nc) as tc:
        with (
            tc.tile_pool(name="wp", bufs=1) as wp,
            tc.tile_pool(name="vp", bufs=3) as vp,
            tc.tile_pool(name="op", bufs=2) as op,
            tc.tile_pool(name="pp", bufs=3, space="PSUM") as pp,
        ):
            wt = wp.tile([128, N_ANG, N_COMB], F32R)
            nc.sync.dma_start(wt[:], w[:])

            for s in range(NSUP):
                l = math.isqrt(s)  # block boundaries fall on perfect squares
                vt_t = vp.tile([128, G * CHUNK], F32R)
                nc.sync.dma_start(
                    vt_t[:], vt[:, s * G * CHUNK:(s + 1) * G * CHUNK])

                ot = op.tile([CHUNK, G, N_COMB], F32)
                for g in range(NGROUP):
                    ps_t = pp.tile([CHUNK, GROUP, N_COMB], F32)
                    for q in range(GROUP):
                        j = g * GROUP + q
                        nc.tensor.matmul(
                            ps_t[:, q, :],
                            vt_t[:, j * CHUNK:(j + 1) * CHUNK],
                            wt[:, l, :],
                            start=True, stop=True)
                    nc.vector.tensor_copy(
                        ot[:, g * GROUP:(g + 1) * GROUP, :], ps_t[:, :, :])

                nc.sync.dma_start(out_v[s], ot[:])

    nc.finalize()  # Bacc compile: wait legalization + reg alloc
    _nc_cache["nc"] = nc
    return nc


def shard_inputs(inputs):
    """Full inputs -> per-core in_maps (host transpose + concat)."""
    w = np.ascontiguousarray(
        np.asarray(inputs["W"], dtype=np.float32).transpose(1, 0, 2))
    in_maps = []
    for i in range(N_CORES):
        vt_i = np.empty((128, ROWS), dtype=np.float32)
        col = 0
        for l in range(N_ANG):
            n = S_CORE * (2 * l + 1)
            blk = np.asarray(inputs[f"values_l{l}"][i * S_CORE:(i + 1) * S_CORE],
                             dtype=np.float32)
            vt_i[:, col:col + n] = blk.reshape(n, 128).T
            col += n
        in_maps.append({"vt": vt_i, "w": w})
    return in_maps


def unshard_output(core_outs):
    """Per-core [90000, 256] -> full [720000, 256]."""
    full = np.empty((N_SAMPLES * M_TOTAL, N_COMB), dtype=np.float32)
    for i, o in enumerate(core_outs):
        for l in range(N_ANG):
            n = S_CORE * (2 * l + 1)
            src0 = S_CORE * l * l                      # local block offset
            dst0 = N_SAMPLES * l * l + i * n           # global block offset
            full[dst0:dst0 + n] = o[src0:src0 + n]
    return full


def run_sharded(in_maps, **kwargs):
    nc = build_nc()
    return run_bass_kernel_spmd(nc, in_maps, core_ids=list(range(N_CORES)),
                                **kwargs)


def kernel(**inputs):
    res = run_sharded(shard_inputs(inputs))
    return unshard_output([r["out"] for r in res.results])
